# revision 26
# baseline (speedup 1.0000x reference)
"""Causal single-head attention (B=8, T=2048, C=768, D=64) on 8 trn2 cores.

Sharding: data-parallel over batch -- one batch element per NeuronCore.
Each core gets x[b] pre-transposed on host to xT layout [4, 6, 128, 512]
(= [t_super, c_chunk, c_within, t_within]) so every on-chip matmul operand
is already in its natural [contraction-on-partitions] layout.

Per-core pipeline (all matmuls in float32r -- full PE rate at N>=256,
~FP22 precision, fp32 PSUM accumulate):
  1. proj pass1: [Wq|Wk] stacked -> psum [qT(0:64); kT(64:128)] per t_super
     proj pass2: [Wv|Wq] stacked -> psum [vT(0:64); qT(64:128)]
     (kT and qT both land on SBUF partitions 64..127 => row-aligned operands
      for the scores matmul; vT on 0..63 feeds PE transposes -> v natural)
  2. v natural [s,64] + ones column -> v' [s,65] (ones col => out' row 64
     accumulates softmax denominators for free)
  3. for each key chunk j (s in [128j,128j+128)): scoresT[s, t>=128j] =
     kT_j^T @ qT (causal block-skip), diagonal triangle masked additively,
     exp(0.125 * x) on ScalarE straight from PSUM -> attn tile,
     out'[65, t] += v'_j^T @ attn_j  (PSUM accumulate over j)
  4. normalize: out[0:64,:] * reciprocal(out[64,:]) broadcast, DMA out.
Output per core: [4, 64, 512] = outT per t_super; host concatenates and
transposes back to [T, D] during unsharding.
"""

import os
import sys

for _p in ("/opt/trn_rl_repo", "/root/.axon_site/_ro/trn_rl_repo"):
    if os.path.isdir(_p) and _p not in sys.path:
        sys.path.append(_p)

import numpy as np

import concourse.bass as bass
import concourse.tile as tile
from concourse import bacc, mybir
from concourse.bass_utils import run_bass_kernel_spmd
from concourse.masks import make_identity

F32 = mybir.dt.float32
F32R = mybir.dt.float32r

B, T, C, D = 8, 2048, 768, 64
NSUP, SUP = 4, 512          # t supers
NCH = 6                     # c chunks of 128
NT128 = T // 128            # 16 key chunks / t blocks
SCALE = float(D) ** -0.5    # 0.125
NEG = -1.0e30

# phases: (supers, n_key_chunks)  -- pair-phasing keeps PSUM within 8 banks
PHASES = (((0, 1), 8), ((2, 3), 16))


def _pieces(t0, t_hi):
    """Split [t0, t_hi) at 512 boundaries -> list of (global_t, n)."""
    out = []
    t = t0
    while t < t_hi:
        n = min(512 - (t % 512), t_hi - t)
        out.append((t, n))
        t += n
    return out


def build_nc():
    nc = bacc.Bacc(None, target_bir_lowering=False)
    xt_d = nc.declare_dram_parameter("xt", [NSUP, NCH, 128, SUP], F32R, isOutput=False)
    w1_d = nc.declare_dram_parameter("w1", [NCH, 128, 128], F32R, isOutput=False)
    w2_d = nc.declare_dram_parameter("w2", [NCH, 128, 128], F32R, isOutput=False)
    mask_d = nc.declare_dram_parameter("mask", [128, 128], F32, isOutput=False)
    out_d = nc.declare_dram_parameter("out", [NSUP, 128, 4, D], F32, isOutput=True)

    with tile.TileContext(nc) as tc:
        with (
            tc.tile_pool(name="consts", bufs=1) as consts,
            tc.tile_pool(name="xpool", bufs=NSUP) as xpool,
            tc.tile_pool(name="ppsum", bufs=2, space="PSUM") as ppsum,
            tc.tile_pool(name="spsum", bufs=2, space="PSUM") as spsum,
            tc.tile_pool(name="opsum", bufs=2, space="PSUM") as opsum,
            tc.tile_pool(name="attnp", bufs=3) as attnp,
            tc.tile_pool(name="outsb", bufs=4) as outsb,
        ):
            w1_sb = consts.tile([128, NCH, 128], F32R)
            w2_sb = consts.tile([128, NCH, 128], F32R)
            mask_sb = consts.tile([128, 128], F32)
            ident = consts.tile([128, 128], F32)
            ident_r = consts.tile([128, 128], F32R)
            # kq_sb: partitions 64:128 = kT ; 0:64 unused
            kq_sb = consts.tile([128, T], F32R)
            # vq_sb: partitions 0:64 = vT ; 64:128 = qT
            vq_sb = consts.tile([128, T], F32R)
            # v natural + ones column, per key chunk j: [128, j, 0:64]=v, [..,64]=1
            vnat = consts.tile([128, NT128, D + 1], F32R)

            nc.sync.dma_start(out=w1_sb, in_=w1_d[:, :, :].transpose([1, 0, 2]))
            nc.sync.dma_start(out=w2_sb, in_=w2_d[:, :, :].transpose([1, 0, 2]))
            nc.sync.dma_start(out=mask_sb, in_=mask_d[:, :])
            make_identity(nc, ident)
            nc.vector.tensor_copy(out=ident_r, in_=ident)
            ones16 = consts.tile([128, NT128, 1], F32)
            nc.vector.memset(ones16, 1.0)
            nc.vector.tensor_copy(out=vnat[:, :, D : D + 1], in_=ones16)

            for supers, njc in PHASES:
                # ---- projections + v-transpose for this phase's supers ----
                for s in supers:
                    xt = xpool.tile([128, NCH, SUP], F32R, tag="xt")
                    nc.sync.dma_start(
                        out=xt, in_=xt_d[s, :, :, :].transpose([1, 0, 2])
                    )
                    p1 = ppsum.tile([128, SUP], F32, tag="proj")
                    for c in range(NCH):
                        nc.tensor.matmul(
                            out=p1,
                            lhsT=w1_sb[:, c, :],
                            rhs=xt[:, c, :],
                            start=(c == 0),
                            stop=(c == NCH - 1),
                        )
                    nc.vector.tensor_copy(
                        out=kq_sb[64:128, s * SUP : (s + 1) * SUP], in_=p1[64:128, :]
                    )
                    p2 = ppsum.tile([128, SUP], F32, tag="proj")
                    for c in range(NCH):
                        nc.tensor.matmul(
                            out=p2,
                            lhsT=w2_sb[:, c, :],
                            rhs=xt[:, c, :],
                            start=(c == 0),
                            stop=(c == NCH - 1),
                        )
                    nc.vector.tensor_copy(
                        out=vq_sb[:, s * SUP : (s + 1) * SUP], in_=p2
                    )
                    # v natural blocks for this super
                    for i in range(4):
                        g = s * 4 + i
                        vt_ps = ppsum.tile([128, D], F32R, tag="proj")
                        nc.tensor.transpose(
                            out=vt_ps,
                            in_=vq_sb[0:64, g * 128 : (g + 1) * 128],
                            identity=ident_r[0:64, 0:64],
                        )
                        nc.vector.tensor_copy(out=vnat[:, g, 0:D], in_=vt_ps)

                # ---- attention over this phase's t range ----
                t_lo = supers[0] * SUP
                t_hi = (supers[-1] + 1) * SUP
                out_ps = {
                    s: opsum.tile([D + 1, SUP], F32, tag="out", name=f"outps{s}")
                    for s in supers
                }
                for j in range(njc):
                    t0 = max(128 * j, t_lo)
                    w = t_hi - t0
                    t_al = (t0 // SUP) * SUP  # bank-align the psum tile base
                    o0 = t0 - t_al
                    sc = spsum.tile([128, t_hi - t_al], F32, tag="scores")
                    for gt, n in _pieces(t0, t_hi):
                        nc.tensor.matmul(
                            out=sc[:, gt - t_al : gt - t_al + n],
                            lhsT=kq_sb[64:128, j * 128 : (j + 1) * 128],
                            rhs=vq_sb[64:128, gt : gt + n],
                            start=True,
                            stop=True,
                        )
                    if t0 == 128 * j:  # diagonal triangle mask
                        nc.vector.tensor_add(
                            out=sc[:, o0 : o0 + 128],
                            in0=sc[:, o0 : o0 + 128],
                            in1=mask_sb,
                        )
                    at = attnp.tile([128, w], F32R, tag="attn")
                    nc.scalar.activation(
                        out=at,
                        in_=sc[:, o0 : o0 + w],
                        func=mybir.ActivationFunctionType.Exp,
                        scale=SCALE,
                    )
                    for gt, n in _pieces(t0, t_hi):
                        s = gt // SUP
                        boff = gt % SUP
                        nc.tensor.matmul(
                            out=out_ps[s][:, boff : boff + n],
                            lhsT=vnat[:, j, :],
                            rhs=at[:, gt - t0 : gt - t0 + n],
                            start=(j == 0),
                            stop=(j == 4 * s + 3),
                        )
                # ---- normalize + store ----
                for s in supers:
                    ou = outsb.tile([D + 1, SUP], F32, tag="ou")
                    nc.vector.tensor_copy(out=ou, in_=out_ps[s])
                    on = outsb.tile([128, 4, D], F32, tag="on")
                    for i in range(4):
                        tr_ps = ppsum.tile([128, D + 1], F32, tag="proj")
                        nc.tensor.transpose(
                            out=tr_ps,
                            in_=ou[:, i * 128 : (i + 1) * 128],
                            identity=ident[0 : D + 1, 0 : D + 1],
                        )
                        rr = outsb.tile([128, 1], F32, tag="rr")
                        nc.vector.reciprocal(out=rr, in_=tr_ps[:, D : D + 1])
                        nc.vector.tensor_scalar_mul(
                            out=on[:, i, :], in0=tr_ps[:, 0:D], scalar1=rr
                        )
                    nc.sync.dma_start(out=out_d[s, :, :, :], in_=on)
    if not nc.is_finalized():
        nc.finalize()
    return nc


def make_in_map(xb, Wq, Wk, Wv):
    """Host-side shard prep for one batch element. xb: [T, C] fp32."""
    # xT [C, T] resliced to [t_super, c_chunk, c_within, t_within]
    xt = np.ascontiguousarray(
        xb.reshape(NSUP, SUP, NCH, 128).transpose(0, 2, 3, 1).astype(np.float32)
    )
    w1 = np.ascontiguousarray(
        np.concatenate([Wq, Wk], axis=1).reshape(NCH, 128, 128).astype(np.float32)
    )
    w2 = np.ascontiguousarray(
        np.concatenate([Wv, Wq], axis=1).reshape(NCH, 128, 128).astype(np.float32)
    )
    ii = np.arange(128)
    mask = np.where(ii[:, None] <= ii[None, :], 0.0, NEG).astype(np.float32)
    return {"xt": xt, "w1": w1, "w2": w2, "mask": mask}


def assemble(results):
    """results: list of per-core out dicts -> full [B, T, D] fp32."""
    outs = []
    for b in range(B):
        o = results[b]["out"]  # [NSUP, 128, 4, D]: t = s*512 + i*128 + p
        outs.append(o.transpose(0, 2, 1, 3).reshape(T, D))
    return np.ascontiguousarray(np.stack(outs)).astype(np.float32)


_NC = None


def _get_nc():
    global _NC
    if _NC is None:
        _NC = build_nc()
    return _NC


def run_hw(inputs, trace=False, **kwargs):
    x = np.asarray(inputs["x"], dtype=np.float32)
    Wq = np.asarray(inputs["Wq"], dtype=np.float32)
    Wk = np.asarray(inputs["Wk"], dtype=np.float32)
    Wv = np.asarray(inputs["Wv"], dtype=np.float32)
    in_maps = [make_in_map(x[b], Wq, Wk, Wv) for b in range(B)]
    nc = _get_nc()
    res = run_bass_kernel_spmd(
        nc, in_maps, core_ids=list(range(B)), trace=trace, **kwargs
    )
    return assemble(res.results), res.exec_time_ns


def kernel(x, Wq, Wk, Wv):
    out, _ = run_hw({"x": x, "Wq": Wq, "Wk": Wk, "Wv": Wv}, trace=False)
    return out


# revision 27
# speedup vs baseline: 1.1353x; 1.1353x over previous
"""Causal single-head attention (B=8, T=2048, C=768, D=64) on 8 trn2 cores.

Sharding: data-parallel over batch -- one batch element per NeuronCore.
Each core gets x[b] pre-transposed on host to xT layout [4, 6, 128, 512]
(= [t_super, c_chunk, c_within, t_within]) so every on-chip matmul operand
is already in its natural [contraction-on-partitions] layout.

Per-core pipeline (all matmuls in float32r -- full PE rate at N>=256,
~FP22 precision, fp32 PSUM accumulate):
  1. proj pass1: [Wq|Wk] stacked -> psum [qT(0:64); kT(64:128)] per t_super
     proj pass2: [Wv|Wq] stacked -> psum [vT(0:64); qT(64:128)]
     (kT and qT both land on SBUF partitions 64..127 => row-aligned operands
      for the scores matmul; vT on 0..63 feeds PE transposes -> v natural)
  2. v natural [s,64] + ones column -> v' [s,65] (ones col => out' row 64
     accumulates softmax denominators for free)
  3. for each key chunk j (s in [128j,128j+128)): scoresT[s, t>=128j] =
     kT_j^T @ qT (causal block-skip), diagonal triangle masked additively,
     exp(0.125 * x) on ScalarE straight from PSUM -> attn tile,
     out'[65, t] += v'_j^T @ attn_j  (PSUM accumulate over j)
  4. normalize: PE-transpose out' 128-col blocks -> [t, 65] natural, DVE
     reciprocal of col 64 + tensor_scalar_mul -> out [t, 64], DMA out.
Output per core: [4, 128, 4, 64] (t = super*512 + block*128 + partition);
host unshard is a pure reshape/transpose.

Emission is software-pipelined for the in-order engines: phase-B
projections are interleaved into phase-A attention, per-super finals are
interleaved into the following attention iterations, and the ACT exp
table is pre-loaded by a dummy activation at kernel start.
"""

import os
import sys

for _p in ("/opt/trn_rl_repo", "/root/.axon_site/_ro/trn_rl_repo"):
    if os.path.isdir(_p) and _p not in sys.path:
        sys.path.append(_p)

import numpy as np

import concourse.bass as bass
import concourse.tile as tile
from concourse import bacc, mybir
from concourse.bass_utils import run_bass_kernel_spmd
from concourse.masks import make_identity

F32 = mybir.dt.float32
F32R = mybir.dt.float32r

B, T, C, D = 8, 2048, 768, 64
NSUP, SUP = 4, 512          # t supers
NCH = 6                     # c chunks of 128
NT128 = T // 128            # 16 key chunks / t blocks
SCALE = float(D) ** -0.5    # 0.125
NEG = -1.0e30

# phases: (supers, n_key_chunks)  -- pair-phasing keeps PSUM within 8 banks
PHASES = (((0, 1), 8), ((2, 3), 16))


def _pieces(t0, t_hi):
    """Split [t0, t_hi) at 512 boundaries -> list of (global_t, n)."""
    out = []
    t = t0
    while t < t_hi:
        n = min(512 - (t % 512), t_hi - t)
        out.append((t, n))
        t += n
    return out


def build_nc():
    nc = bacc.Bacc(None, target_bir_lowering=False)
    xt_d = nc.declare_dram_parameter("xt", [NSUP, NCH, 128, SUP], F32R, isOutput=False)
    w1_d = nc.declare_dram_parameter("w1", [NCH, 128, 128], F32R, isOutput=False)
    w2_d = nc.declare_dram_parameter("w2", [NCH, 128, 128], F32R, isOutput=False)
    mask_d = nc.declare_dram_parameter("mask", [128, 128], F32, isOutput=False)
    out_d = nc.declare_dram_parameter("out", [NSUP, 128, 4, D], F32, isOutput=True)

    with tile.TileContext(nc) as tc:
        with (
            tc.tile_pool(name="consts", bufs=1) as consts,
            tc.tile_pool(name="xpool", bufs=NSUP) as xpool,
            tc.tile_pool(name="ppsum", bufs=2, space="PSUM") as ppsum,
            tc.tile_pool(name="spsum", bufs=2, space="PSUM") as spsum,
            tc.tile_pool(name="opsum", bufs=2, space="PSUM") as opsum,
            tc.tile_pool(name="attnp", bufs=4) as attnp,
            tc.tile_pool(name="outsb", bufs=4) as outsb,
        ):
            w1_sb = consts.tile([128, NCH, 128], F32R)
            w2_sb = consts.tile([128, NCH, 128], F32R)
            mask_sb = consts.tile([128, 128], F32)
            ident = consts.tile([128, 128], F32)
            ident_r = consts.tile([128, 128], F32R)
            # kq_sb: partitions 64:128 = kT ; 0:64 unused
            kq_sb = consts.tile([128, T], F32R)
            # vq_sb: partitions 0:64 = vT ; 64:128 = qT
            vq_sb = consts.tile([128, T], F32R)
            # v natural + ones column, per key chunk j: [128, j, 0:64]=v, [..,64]=1
            vnat = consts.tile([128, NT128, D + 1], F32R)
            ones16 = consts.tile([128, NT128, 1], F32)
            scratch = consts.tile([1, 1], F32)

            # pre-load the ACT exp table while input DMAs stream
            nc.vector.memset(ones16, 1.0)
            nc.scalar.activation(
                out=scratch,
                in_=ones16[0:1, 0, :],
                func=mybir.ActivationFunctionType.Exp,
            )

            # input DMAs in consumption order (sync engine is in-order)
            nc.sync.dma_start(out=w1_sb, in_=w1_d[:, :, :].transpose([1, 0, 2]))
            nc.sync.dma_start(out=w2_sb, in_=w2_d[:, :, :].transpose([1, 0, 2]))
            nc.sync.dma_start(out=mask_sb, in_=mask_d[:, :])

            xts = {}

            def emit_xt_dma(s):
                xt = xpool.tile([128, NCH, SUP], F32R, tag="xt", name=f"xt{s}")
                h = NCH // 2
                nc.sync.dma_start(
                    out=xt[:, 0:h, :], in_=xt_d[s, 0:h, :, :].transpose([1, 0, 2])
                )
                nc.sync.dma_start(
                    out=xt[:, h:NCH, :], in_=xt_d[s, h:NCH, :, :].transpose([1, 0, 2])
                )
                xts[s] = xt

            def proj_units(s):
                """Projection work for one super as a list of thunks."""
                xt = xts[s]

                def pass1():
                    p1 = ppsum.tile([128, SUP], F32, tag="proj", name=f"p1_{s}")
                    for c in range(NCH):
                        nc.tensor.matmul(
                            out=p1,
                            lhsT=w1_sb[:, c, :],
                            rhs=xt[:, c, :],
                            start=(c == 0),
                            stop=(c == NCH - 1),
                        )
                    nc.vector.tensor_copy(
                        out=kq_sb[64:128, s * SUP : (s + 1) * SUP], in_=p1[64:128, :]
                    )

                def pass2():
                    p2 = ppsum.tile([128, SUP], F32, tag="proj", name=f"p2_{s}")
                    for c in range(NCH):
                        nc.tensor.matmul(
                            out=p2,
                            lhsT=w2_sb[:, c, :],
                            rhs=xt[:, c, :],
                            start=(c == 0),
                            stop=(c == NCH - 1),
                        )
                    nc.vector.tensor_copy(
                        out=vq_sb[:, s * SUP : (s + 1) * SUP], in_=p2
                    )

                def vtrans():
                    for i in range(4):
                        g = s * 4 + i
                        vt_ps = ppsum.tile(
                            [128, D], F32R, tag="proj", name=f"vt{g}"
                        )
                        nc.tensor.transpose(
                            out=vt_ps,
                            in_=vq_sb[0:64, g * 128 : (g + 1) * 128],
                            identity=ident_r[0:64, 0:64],
                        )
                        nc.vector.tensor_copy(out=vnat[:, g, 0:D], in_=vt_ps)

                return [pass1, pass2, vtrans]

            def emit_finals(s):
                """Normalize + store one finished t_super."""
                ou = outsb.tile([D + 1, SUP], F32, tag="ou", name=f"ou{s}")
                nc.vector.tensor_copy(out=ou, in_=out_ps[s])
                on = outsb.tile([128, 4, D], F32, tag="on", name=f"on{s}")
                for i in range(4):
                    tr_ps = ppsum.tile([128, D + 1], F32, tag="proj", name=f"tr{s}_{i}")
                    nc.tensor.transpose(
                        out=tr_ps,
                        in_=ou[:, i * 128 : (i + 1) * 128],
                        identity=ident[0 : D + 1, 0 : D + 1],
                    )
                    rr = outsb.tile([128, 1], F32, tag="rr", name=f"rr{s}_{i}")
                    nc.vector.reciprocal(out=rr, in_=tr_ps[:, D : D + 1])
                    nc.vector.tensor_scalar_mul(
                        out=on[:, i, :], in0=tr_ps[:, 0:D], scalar1=rr
                    )
                nc.sync.dma_start(out=out_d[s, :, :, :], in_=on)

            def attn_iter(j, t_lo, t_hi, supers):
                t0 = max(128 * j, t_lo)
                w = t_hi - t0
                t_al = (t0 // SUP) * SUP  # bank-align the psum tile base
                o0 = t0 - t_al
                sc = spsum.tile([128, t_hi - t_al], F32, tag="scores", name=f"sc{j}")
                first = True
                for gt, n in _pieces(t0, t_hi):
                    nc.tensor.matmul(
                        out=sc[:, gt - t_al : gt - t_al + n],
                        lhsT=kq_sb[64:128, j * 128 : (j + 1) * 128],
                        rhs=vq_sb[64:128, gt : gt + n],
                        start=True,
                        stop=True,
                    )
                    if first and t0 == 128 * j:  # diagonal triangle mask
                        nc.vector.tensor_add(
                            out=sc[:, o0 : o0 + 128],
                            in0=sc[:, o0 : o0 + 128],
                            in1=mask_sb,
                        )
                    first = False
                at = attnp.tile([128, w], F32R, tag="attn", name=f"at{j}")
                nc.scalar.activation(
                    out=at,
                    in_=sc[:, o0 : o0 + w],
                    func=mybir.ActivationFunctionType.Exp,
                    scale=SCALE,
                )
                for gt, n in _pieces(t0, t_hi):
                    s = gt // SUP
                    boff = gt % SUP
                    nc.tensor.matmul(
                        out=out_ps[s][:, boff : boff + n],
                        lhsT=vnat[:, j, :],
                        rhs=at[:, gt - t0 : gt - t0 + n],
                        start=(j == 0),
                        stop=(j == 4 * s + 3),
                    )

            # ---------------- phase A ----------------
            make_identity(nc, ident)
            nc.vector.tensor_copy(out=ident_r, in_=ident)
            nc.vector.tensor_copy(out=vnat[:, :, D : D + 1], in_=ones16)

            emit_xt_dma(0)
            emit_xt_dma(1)
            for s in (0, 1):
                for u in proj_units(s):
                    u()
            emit_xt_dma(2)
            emit_xt_dma(3)

            # phase-B projections, interleaved into phase-A attention below
            fillers = {2: proj_units(2)[0], 3: proj_units(2)[1],
                       4: proj_units(2)[2], 5: proj_units(3)[0],
                       6: proj_units(3)[1], 7: proj_units(3)[2]}

            out_ps = {
                s: opsum.tile([D + 1, SUP], F32, tag="out", name=f"outps{s}")
                for s in (0, 1)
            }
            for j in range(8):
                attn_iter(j, 0, 1024, (0, 1))
                if j in fillers:
                    fillers[j]()
            finals_a = {0: 0, 2: 1}  # B-iteration -> phase-A super

            # ---------------- phase B ----------------
            out_ps = dict(out_ps)  # keep A refs for interleaved finals
            out_ps.update(
                {
                    s: opsum.tile([D + 1, SUP], F32, tag="out", name=f"outps{s}")
                    for s in (2, 3)
                }
            )
            for j in range(16):
                attn_iter(j, 1024, 2048, (2, 3))
                if j in finals_a:
                    emit_finals(finals_a[j])
                if j == 12:
                    emit_finals(2)
            emit_finals(3)
    if not nc.is_finalized():
        nc.finalize()
    return nc


def make_in_map(xb, Wq, Wk, Wv):
    """Host-side shard prep for one batch element. xb: [T, C] fp32."""
    # xT [C, T] resliced to [t_super, c_chunk, c_within, t_within]
    xt = np.ascontiguousarray(
        xb.reshape(NSUP, SUP, NCH, 128).transpose(0, 2, 3, 1).astype(np.float32)
    )
    w1 = np.ascontiguousarray(
        np.concatenate([Wq, Wk], axis=1).reshape(NCH, 128, 128).astype(np.float32)
    )
    w2 = np.ascontiguousarray(
        np.concatenate([Wv, Wq], axis=1).reshape(NCH, 128, 128).astype(np.float32)
    )
    ii = np.arange(128)
    mask = np.where(ii[:, None] <= ii[None, :], 0.0, NEG).astype(np.float32)
    return {"xt": xt, "w1": w1, "w2": w2, "mask": mask}


def assemble(results):
    """results: list of per-core out dicts -> full [B, T, D] fp32."""
    outs = []
    for b in range(B):
        o = results[b]["out"]  # [NSUP, 128, 4, D]: t = s*512 + i*128 + p
        outs.append(o.transpose(0, 2, 1, 3).reshape(T, D))
    return np.ascontiguousarray(np.stack(outs)).astype(np.float32)


_NC = None


def _get_nc():
    global _NC
    if _NC is None:
        _NC = build_nc()
    return _NC


def run_hw(inputs, trace=False, **kwargs):
    x = np.asarray(inputs["x"], dtype=np.float32)
    Wq = np.asarray(inputs["Wq"], dtype=np.float32)
    Wk = np.asarray(inputs["Wk"], dtype=np.float32)
    Wv = np.asarray(inputs["Wv"], dtype=np.float32)
    in_maps = [make_in_map(x[b], Wq, Wk, Wv) for b in range(B)]
    nc = _get_nc()
    res = run_bass_kernel_spmd(
        nc, in_maps, core_ids=list(range(B)), trace=trace, **kwargs
    )
    return assemble(res.results), res.exec_time_ns


def kernel(x, Wq, Wk, Wv):
    out, _ = run_hw({"x": x, "Wq": Wq, "Wk": Wk, "Wv": Wv}, trace=False)
    return out


# revision 34
# speedup vs baseline: 1.3361x; 1.1768x over previous
"""Causal single-head attention (B=8, T=2048, C=768, D=64) on 8 trn2 cores.

Sharding: data-parallel over batch -- one batch element per NeuronCore.
Each core gets x[b] pre-transposed on host to xT layout [4, 6, 128, 512]
(= [t_super, c_chunk, c_within, t_within]) so every on-chip matmul operand
is already in its natural [contraction-on-partitions] layout.

Per-core pipeline (all matmuls in float32r -- full PE rate at N>=256,
~FP22 precision, fp32 PSUM accumulate):
  1. proj pass1: [Wq|Wk] stacked -> psum [qT(0:64); kT(64:128)] per t_super
     proj pass2: [Wv|Wq] stacked -> psum [vT(0:64); qT(64:128)]
     (kT and qT both land on SBUF partitions 64..127 => row-aligned operands
      for the scores matmul; vT on 0..63 feeds PE transposes -> v natural)
  2. v natural [s,64] + ones column -> v' [s,65] (ones col => out' row 64
     accumulates softmax denominators for free)
  3. for each key chunk j (s in [128j,128j+128)): scoresT[s, t>=128j] =
     kT_j^T @ qT (causal block-skip), diagonal triangle masked additively,
     exp(0.125 * x) on ScalarE straight from PSUM -> attn tile,
     out'[65, t] += v'_j^T @ attn_j  (PSUM accumulate over j)
  4. normalize: PE-transpose out' 128-col blocks -> [t, 65] natural, DVE
     reciprocal of col 64 + tensor_scalar_mul -> out [t, 64], DMA out.
Output per core: [4, 128, 4, 64] (t = super*512 + block*128 + partition);
host unshard is a pure reshape/transpose.

Emission is software-pipelined for the in-order engines: phase-B
projections are interleaved into phase-A attention, per-super finals are
interleaved into the following attention iterations, and the ACT exp
table is pre-loaded by a dummy activation at kernel start.
"""

import os
import sys

for _p in ("/opt/trn_rl_repo", "/root/.axon_site/_ro/trn_rl_repo"):
    if os.path.isdir(_p) and _p not in sys.path:
        sys.path.append(_p)

import numpy as np

import concourse.bass as bass
import concourse.tile as tile
from concourse import bacc, mybir
from concourse.bass_utils import run_bass_kernel_spmd
from concourse.masks import make_identity

F32 = mybir.dt.float32
F32R = mybir.dt.float32r

B, T, C, D = 8, 2048, 768, 64
NSUP, SUP = 4, 512          # t supers
NCH = 6                     # c chunks of 128
NT128 = T // 128            # 16 key chunks / t blocks
SCALE = float(D) ** -0.5    # 0.125
NEG = -1.0e30

# phases: (supers, n_key_chunks)  -- pair-phasing keeps PSUM within 8 banks
PHASES = (((0, 1), 8), ((2, 3), 16))


def _pieces(t0, t_hi):
    """Split [t0, t_hi) at 512 boundaries -> list of (global_t, n)."""
    out = []
    t = t0
    while t < t_hi:
        n = min(512 - (t % 512), t_hi - t)
        out.append((t, n))
        t += n
    return out


def build_nc():
    nc = bacc.Bacc(None, target_bir_lowering=False)
    xt_d = nc.declare_dram_parameter("xt", [NSUP, NCH, 128, SUP], F32R, isOutput=False)
    w1_d = nc.declare_dram_parameter("w1", [NCH, 128, 128], F32R, isOutput=False)
    w2_d = nc.declare_dram_parameter("w2", [NCH, 128, 128], F32R, isOutput=False)
    mask_d = nc.declare_dram_parameter("mask", [128, 128], F32, isOutput=False)
    out_d = nc.declare_dram_parameter("out", [NSUP, 128, 4, D], F32, isOutput=True)

    with tile.TileContext(nc) as tc:
        with (
            tc.tile_pool(name="consts", bufs=1) as consts,
            tc.tile_pool(name="xpool", bufs=NSUP) as xpool,
            tc.tile_pool(name="ppsum", bufs=2, space="PSUM") as ppsum,
            tc.tile_pool(name="spsum", bufs=4, space="PSUM") as spsum,
            tc.tile_pool(name="opsum", bufs=2, space="PSUM") as opsum,
            tc.tile_pool(name="attnp", bufs=6) as attnp,
            tc.tile_pool(name="outsb", bufs=4) as outsb,
        ):
            w1_sb = consts.tile([128, NCH, 128], F32R)
            w2_sb = consts.tile([128, NCH, 128], F32R)
            mask_sb = consts.tile([128, 128], F32)
            ident = consts.tile([128, 128], F32)
            ident_r = consts.tile([128, 128], F32R)
            # kq_sb: partitions 64:128 = kT ; 0:64 unused
            kq_sb = consts.tile([128, T], F32R)
            # vq_sb: partitions 0:64 = vT ; 64:128 = qT
            vq_sb = consts.tile([128, T], F32R)
            # v natural + ones column, per key chunk j: [128, j, 0:64]=v, [..,64]=1
            vnat = consts.tile([128, NT128, D + 1], F32R)
            ones16 = consts.tile([128, NT128, 1], F32)
            scratch = consts.tile([1, 1], F32)

            # pre-load the ACT exp table while input DMAs stream
            nc.vector.memset(ones16, 1.0)
            nc.scalar.activation(
                out=scratch,
                in_=ones16[0:1, 0, :],
                func=mybir.ActivationFunctionType.Exp,
            )

            xts = {}

            H = NCH // 2

            def emit_xt_dma(s, half=None):
                if s not in xts:
                    xts[s] = (
                        xpool.tile([128, H, SUP], F32R, tag="xta", name=f"xta{s}"),
                        xpool.tile([128, H, SUP], F32R, tag="xtb", name=f"xtb{s}"),
                    )
                halves = (0, 1) if half is None else (half,)
                for h in halves:
                    nc.sync.dma_start(
                        out=xts[s][h],
                        in_=xt_d[s, h * H : (h + 1) * H, :, :].transpose([1, 0, 2]),
                    )

            def proj_units(s):
                """Projection work for one super as a list of thunks."""
                xa, xb = xts[s]

                def one_pass(w_sb, nm):
                    pp = ppsum.tile([128, SUP], F32, tag="proj", name=f"{nm}_{s}")
                    for c in range(NCH):
                        nc.tensor.matmul(
                            out=pp,
                            lhsT=w_sb[:, c, :],
                            rhs=(xa if c < H else xb)[:, c % H, :],
                            start=(c == 0),
                            stop=(c == NCH - 1),
                        )
                    return pp

                def pass1():
                    p1 = one_pass(w1_sb, "p1")
                    nc.vector.tensor_copy(
                        out=kq_sb[64:128, s * SUP : (s + 1) * SUP], in_=p1[64:128, :]
                    )

                def pass2():
                    p2 = one_pass(w2_sb, "p2")
                    nc.vector.tensor_copy(
                        out=vq_sb[:, s * SUP : (s + 1) * SUP], in_=p2
                    )

                def vtrans():
                    for i in range(4):
                        g = s * 4 + i
                        vt_ps = ppsum.tile(
                            [128, D], F32R, tag="proj", name=f"vt{g}"
                        )
                        nc.tensor.transpose(
                            out=vt_ps,
                            in_=vq_sb[0:64, g * 128 : (g + 1) * 128],
                            identity=ident_r[0:64, 0:64],
                        )
                        nc.vector.tensor_copy(out=vnat[:, g, 0:D], in_=vt_ps)

                return [pass1, pass2, vtrans]

            def emit_finals(s):
                """Normalize + store one finished t_super."""
                ou = outsb.tile([D + 1, SUP], F32, tag="ou", name=f"ou{s}")
                nc.vector.tensor_copy(out=ou, in_=out_ps[s])
                on = outsb.tile([128, 4, D], F32, tag="on", name=f"on{s}")
                for i in range(4):
                    tr_ps = ppsum.tile([128, D + 1], F32, tag="proj", name=f"tr{s}_{i}")
                    nc.tensor.transpose(
                        out=tr_ps,
                        in_=ou[:, i * 128 : (i + 1) * 128],
                        identity=ident[0 : D + 1, 0 : D + 1],
                    )
                    rr = outsb.tile([128, 1], F32, tag="rr", name=f"rr{s}_{i}")
                    nc.vector.reciprocal(out=rr, in_=tr_ps[:, D : D + 1])
                    nc.vector.tensor_scalar_mul(
                        out=on[:, i, :], in0=tr_ps[:, 0:D], scalar1=rr
                    )
                nc.sync.dma_start(out=out_d[s, :, :, :], in_=on)

            def attn_iter(j, t_lo, t_hi, supers):
                t0 = max(128 * j, t_lo)
                # one scores psum tile + exp + out-accumulate per 512-piece:
                # single-bank tiles, 4-deep pipelining across pieces
                for pi, (gt, n) in enumerate(_pieces(t0, t_hi)):
                    sc = spsum.tile([128, n], F32, tag="scores", name=f"sc{j}_{pi}")
                    nc.tensor.matmul(
                        out=sc,
                        lhsT=kq_sb[64:128, j * 128 : (j + 1) * 128],
                        rhs=vq_sb[64:128, gt : gt + n],
                        start=True,
                        stop=True,
                    )
                    if pi == 0 and t0 == 128 * j:  # diagonal triangle mask
                        nc.vector.tensor_add(
                            out=sc[:, 0:128], in0=sc[:, 0:128], in1=mask_sb
                        )
                    at = attnp.tile([128, n], F32R, tag="attn", name=f"at{j}_{pi}")
                    nc.scalar.activation(
                        out=at,
                        in_=sc,
                        func=mybir.ActivationFunctionType.Exp,
                        scale=SCALE,
                    )
                    s = gt // SUP
                    boff = gt % SUP
                    nc.tensor.matmul(
                        out=out_ps[s][:, boff : boff + n],
                        lhsT=vnat[:, j, :],
                        rhs=at,
                        start=(j == 0),
                        stop=(j == 4 * s + 3),
                    )

            # ---------------- phase A ----------------
            make_identity(nc, ident)
            nc.vector.tensor_copy(out=ident_r, in_=ident)
            nc.vector.tensor_copy(out=vnat[:, :, D : D + 1], in_=ones16)

            # input DMAs in consumption order (sync engine is in-order)
            nc.sync.dma_start(out=w1_sb, in_=w1_d[:, :, :].transpose([1, 0, 2]))
            emit_xt_dma(0, half=0)
            emit_xt_dma(0, half=1)
            nc.sync.dma_start(out=w2_sb, in_=w2_d[:, :, :].transpose([1, 0, 2]))
            emit_xt_dma(1, half=0)
            emit_xt_dma(1, half=1)
            nc.sync.dma_start(out=mask_sb, in_=mask_d[:, :])
            for s in (0, 1):
                for u in proj_units(s):
                    u()
            emit_xt_dma(2)
            emit_xt_dma(3)

            # phase-B projections, interleaved into phase-A attention below
            fillers = {2: proj_units(2)[0], 3: proj_units(2)[1],
                       4: proj_units(2)[2], 5: proj_units(3)[0],
                       6: proj_units(3)[1], 7: proj_units(3)[2]}

            out_ps = {
                s: opsum.tile([D + 1, SUP], F32, tag="out", name=f"outps{s}")
                for s in (0, 1)
            }
            for j in range(8):
                attn_iter(j, 0, 1024, (0, 1))
                if j in fillers:
                    fillers[j]()
            finals_a = {0: 0, 2: 1}  # B-iteration -> phase-A super

            # ---------------- phase B ----------------
            out_ps = dict(out_ps)  # keep A refs for interleaved finals
            out_ps.update(
                {
                    s: opsum.tile([D + 1, SUP], F32, tag="out", name=f"outps{s}")
                    for s in (2, 3)
                }
            )
            for j in range(16):
                attn_iter(j, 1024, 2048, (2, 3))
                if j in finals_a:
                    emit_finals(finals_a[j])
                if j == 12:
                    emit_finals(2)
            emit_finals(3)
    if not nc.is_finalized():
        nc.finalize()
    return nc


def make_in_map(xb, Wq, Wk, Wv):
    """Host-side shard prep for one batch element. xb: [T, C] fp32."""
    # xT [C, T] resliced to [t_super, c_chunk, c_within, t_within]
    xt = np.ascontiguousarray(
        xb.reshape(NSUP, SUP, NCH, 128).transpose(0, 2, 3, 1).astype(np.float32)
    )
    w1 = np.ascontiguousarray(
        np.concatenate([Wq, Wk], axis=1).reshape(NCH, 128, 128).astype(np.float32)
    )
    w2 = np.ascontiguousarray(
        np.concatenate([Wv, Wq], axis=1).reshape(NCH, 128, 128).astype(np.float32)
    )
    ii = np.arange(128)
    mask = np.where(ii[:, None] <= ii[None, :], 0.0, NEG).astype(np.float32)
    return {"xt": xt, "w1": w1, "w2": w2, "mask": mask}


def assemble(results):
    """results: list of per-core out dicts -> full [B, T, D] fp32."""
    outs = []
    for b in range(B):
        o = results[b]["out"]  # [NSUP, 128, 4, D]: t = s*512 + i*128 + p
        outs.append(o.transpose(0, 2, 1, 3).reshape(T, D))
    return np.ascontiguousarray(np.stack(outs)).astype(np.float32)


_NC = None


def _get_nc():
    global _NC
    if _NC is None:
        _NC = build_nc()
    return _NC


def run_hw(inputs, trace=False, **kwargs):
    x = np.asarray(inputs["x"], dtype=np.float32)
    Wq = np.asarray(inputs["Wq"], dtype=np.float32)
    Wk = np.asarray(inputs["Wk"], dtype=np.float32)
    Wv = np.asarray(inputs["Wv"], dtype=np.float32)
    in_maps = [make_in_map(x[b], Wq, Wk, Wv) for b in range(B)]
    nc = _get_nc()
    res = run_bass_kernel_spmd(
        nc, in_maps, core_ids=list(range(B)), trace=trace, **kwargs
    )
    return assemble(res.results), res.exec_time_ns


def kernel(x, Wq, Wk, Wv):
    out, _ = run_hw({"x": x, "Wq": Wq, "Wk": Wk, "Wv": Wv}, trace=False)
    return out
